# revision 14
# baseline (speedup 1.0000x reference)
"""DGCNN (nn_DGCNN_32727650795899) Trainium2 Bass kernel.

Sharding: B=4 samples x 2 row-halves -> 8 cores. Core c handles sample c//2,
point rows [ (c%2)*2048, (c%2+1)*2048 ). Weights replicated. Pairs of cores
exchange x1/x2 feature halves via AllGather and the global-max vector via
AllReduce(max).

Numerics: edge layers run exact fp32 (selection parity with the reference's
top-k); head convs run float32r.

Key perf structure vs the v0 kernel (measured on HW):
- topk: Max/MatchReplace cost ~4-5x more at free=4096 than at 2048, so each
  of the 3 rounds runs Max8 and MatchReplace split on the two 2048-halves
  (the global top-8 per round is Max8 of the 16 concatenated half-maxima).
  MaxIndex runs full-width once per round (split is not cheaper for it), so
  indices are global first-occurrence — identical tie semantics to
  jax.lax.top_k and to the v0 kernel.
- gather: the Q7 ap_gather cost is per-index per-core, so A is duplicated to
  partitions 64:127 (free via a [Cin,128] doubled lhsT) and the two 64-row
  halves of each 128-row block gather concurrently on all 8 Q7 cores with
  num_idxs=1280 ("dual-half" layout: partition c+64h = channel c, row-half h).
  All post-gather stages (ttadd/prelu/W2-matmul/max-over-k) run dual-half,
  halving their free size.
- the block loop emits stage-shifted (pd(t+1), topk(t), gather(t), tail(t-1),
  reduce(t-2)) so DVE topk, Pool gather, ACT copies and PE matmuls of
  adjacent blocks overlap instead of serializing.

Self-contained: hardcodes all shapes; builds/compiles the Bass program on
first call and runs it on NeuronCores 0-7 via PJRT.
"""

import contextlib
import sys

sys.path.insert(0, "/opt/trn_rl_repo")

import numpy as np

from concourse import bacc, mybir, tile

FP32 = mybir.dt.float32
F32R = mybir.dt.float32r
U32 = mybir.dt.uint32
U16 = mybir.dt.uint16
I16 = mybir.dt.int16
ACT = mybir.ActivationFunctionType
ALU = mybir.AluOpType

B = 4
C0 = 9
N = 4096
HALF = N // 2          # rows per core
NBLK = HALF // 128     # 16 row blocks per core
KNN = 20
EPS = 1e-5
NEG = -3.0e38
GGRP = 2               # blocks per ap_gather (amortizes Q7 fixed cost)

# packed bias tile columns: (offset, width); b1..b5 are dual-stacked [128]
BIAS_LAYOUT = {
    "b1": (0, 1), "b2": (1, 1), "b3": (2, 1), "b4": (3, 1), "b5": (4, 1),
    "b6": (5, 8), "b7": (13, 4), "b8": (17, 2),
}
BIAS_W = 19
# packed weights tile: name -> (col offset, rows, cols)
WPACK = {
    "wnW1": (0, 9, 128), "bw2_1": (128, 18, 128), "w2T2": (256, 128, 128),
    "wnW3": (384, 64, 128), "bw2_3": (512, 128, 128), "w4T2": (640, 128, 128),
    "wnW5": (768, 64, 128), "bw2_5": (896, 128, 128), "ones": (1024, 64, 1),
}
WPACK_W = 1025

_CACHE = {}
ABLATE = set()  # debug ablations: 'topk','gather'


def _edge_layer(nc, tc, pools, Cin, xmy_aug, xmy2, xf, wnW, bw2, ones_col,
                b_in2, w2T2, b_out2, xmy2_next, y_full, lname):
    """One EdgeConv block over this core's 2048 rows.

    xmy_aug: [Cin+1, HALF] full layout (pd lhsT; row Cin = ones)
    xmy2:    [2*Cin, 1024] dual-half layout (Bt2 rhs)
    xf:      [65, N] fp32; rows 0:Cin hold 2x full features; this layer
             writes row Cin = -|x_j|^2, then pd = xmy_aug^T @ xf[0:Cin+1].
    y_full:  [.., HALF] full-layout output rows 0:64 (written via 2 DMAs/blk)
    xmy2_next: [128, 1024] dual-half output (None for layer 3)
    """
    pool, ppool = pools
    lctx = contextlib.ExitStack()
    lpool = lctx.enter_context(tc.tile_pool(name=f"L{lname}", bufs=1))
    zpool = None
    if w2T2 is not None:
        zpool = lctx.enter_context(
            tc.tile_pool(name=f"Lz{lname}", bufs=1, space="PSUM"))

    # ---- per-layer precompute over the full 4096 columns ----
    # xf row Cin := -|x|^2 = -0.25 * sum((2x)^2)
    for ch in range(8):
        sl = slice(ch * 512, (ch + 1) * 512)
        xsq = pool.tile([Cin, 512], FP32, tag="xsq", bufs=2,
                        name=f"xsq_{lname}_{ch}")
        nc.scalar.activation(out=xsq[:], in_=xf[0:Cin, sl], func=ACT.Square)
        pp = ppool.tile([128, 512], FP32, tag="mm", name=f"ppxx_{lname}_{ch}")
        nc.tensor.matmul(pp[0:1, :], ones_col, xsq[:], start=True, stop=True)
        xxn = pool.tile([1, 512], FP32, tag="xxn", bufs=2,
                        name=f"xxn_{lname}_{ch}")
        nc.scalar.activation(out=xxn[:], in_=pp[0:1, :], func=ACT.Copy,
                             scale=-0.25)
        # aug row of xf (DMA; compute engines can't start at partition Cin)
        nc.sync.dma_start(xf[Cin:Cin + 1, sl], xxn[:])

    # A[c, j] = A[c+64, j] = ((Wn/2)^T-fold @ 2 xf)[c, j]  (doubled lhsT)
    A = lpool.tile([128, N], FP32, name=f"A_{lname}")
    for ch in range(8):
        sl = slice(ch * 512, (ch + 1) * 512)
        pp = ppool.tile([128, 512], FP32, tag="mm", name=f"ppA_{lname}_{ch}")
        nc.tensor.matmul(pp[:], wnW, xf[0:Cin, sl], start=True, stop=True)
        nc.scalar.activation(out=A[:, sl], in_=pp[:], func=ACT.Copy)
    # Bt2[c+64h, blk*64+q] = (Bw @ x_my)[c, blk*128 + 64h + q]  (block-diag)
    Bt2 = lpool.tile([128, HALF // 2], FP32, name=f"B_{lname}")
    for ch in range(2):
        sl = slice(ch * 512, (ch + 1) * 512)
        pp = ppool.tile([128, 512], FP32, tag="mm", name=f"ppB_{lname}_{ch}")
        nc.tensor.matmul(pp[:], bw2, xmy2[0:2 * Cin, sl], start=True,
                         stop=True)
        nc.scalar.activation(out=Bt2[:, sl], in_=pp[:], func=ACT.Copy)

    # ---- per 128-row block stages ----
    pd_t, kidx_t, idxr_t, E_t, h_t, z2_t, yd_t = {}, {}, {}, {}, {}, {}, {}

    def s_pd(b):
        rsl = slice(b * 128, (b + 1) * 128)
        pd = pool.tile([128, N], FP32, tag="pd", bufs=2, name=f"pd_{lname}_{b}")
        pd_t[b] = pd
        for ch in range(8):
            sl = slice(ch * 512, (ch + 1) * 512)
            pp = ppool.tile([128, 512], FP32, tag="mm",
                            name=f"ppd_{lname}_{b}_{ch}")
            nc.tensor.matmul(pp[:], xmy_aug[:, rsl], xf[0:Cin + 1, sl],
                             start=True, stop=True)
            nc.scalar.activation(out=pd[:, sl], in_=pp[:], func=ACT.Copy)

    def s_topk(b):
        pd = pd_t.pop(b)
        L = pd[:, 0:2048]
        R = pd[:, 2048:4096]
        v16 = pool.tile([128, 16], FP32, tag="v16", bufs=2,
                        name=f"v16_{lname}_{b}")
        m24 = pool.tile([128, 24], FP32, tag="m24", bufs=2,
                        name=f"m24_{lname}_{b}")
        kidx = pool.tile([128, 24], U16, tag="kidx", bufs=2,
                         name=f"kidx_{lname}_{b}")
        kidx_t[b] = kidx
        if "topk" not in ABLATE:
            for r in range(3):
                s8 = slice(r * 8, (r + 1) * 8)
                m8 = m24[:, s8]
                nc.vector.max(v16[:, 0:8], L)
                nc.vector.max(v16[:, 8:16], R)
                nc.vector.max(m8, v16[:])
                # full-width index search: global indices, first occurrence
                nc.vector.max_index(kidx[:, s8], m8, pd[:])
                if r < 2:
                    nc.vector.match_replace(L, m8, L, NEG)
                    nc.vector.match_replace(R, m8, R, NEG)
        else:
            nc.vector.memset(kidx[:].bitcast(FP32), 0)
        # wrap + replicate indices: core 16c..16c+15 needs its half's list.
        # Blocks are gathered in groups of GGRP (amortizes the Q7 fixed
        # cost): block b writes the (b%GGRP)-th 80-col group of the group's
        # index tile.
        if b % GGRP == 0:
            idxr = pool.tile([128, 80 * GGRP], U16, tag="idxr", bufs=2,
                             name=f"idxr_{lname}_{b}")
            idxr_t[b // GGRP] = idxr
        else:
            idxr = idxr_t[b // GGRP]
        co = 80 * (b % GGRP)
        csl = slice(co, co + 80)
        qengs = [nc.sync, nc.scalar]
        for h in range(2):
            pb = 64 * h
            for g in range(4):
                qengs[g % 2].dma_start(
                    idxr[pb:pb + 16, co + g * 20:co + (g + 1) * 20],
                    kidx[pb + 16 * g:pb + 16 * (g + 1), 0:20])
        # depth-1 replication (all copies read the freshly-wrapped 16 rows)
        nc.sync.dma_start(idxr[16:32, csl], idxr[0:16, csl])
        nc.scalar.dma_start(idxr[32:48, csl], idxr[0:16, csl])
        nc.sync.dma_start(idxr[48:64, csl], idxr[0:16, csl])
        nc.scalar.dma_start(idxr[80:96, csl], idxr[64:80, csl])
        nc.sync.dma_start(idxr[96:112, csl], idxr[64:80, csl])
        nc.scalar.dma_start(idxr[112:128, csl], idxr[64:80, csl])

    def s_gth(p):
        # gather blocks (GGRP*p .. GGRP*p + GGRP-1) in one Q7 instruction
        idxr = idxr_t.pop(p)
        E = pool.tile([128, GGRP * 64 * KNN], FP32, tag="E", bufs=2,
                      name=f"E_{lname}_{p}")
        for j in range(GGRP):
            E_t[GGRP * p + j] = E[:, j * 64 * KNN:(j + 1) * 64 * KNN]
        if "gather" not in ABLATE:
            nc.gpsimd.ap_gather(
                E[:].unsqueeze(-1), A[:].unsqueeze(-1), idxr[:].bitcast(I16),
                channels=128, num_elems=N, d=1, num_idxs=GGRP * 64 * KNN)
        else:
            nc.vector.memset(E[:], 0.0)

    def bt_slice(b):
        return Bt2[:, b * 64:(b + 1) * 64] \
            .rearrange("c (g r) -> c g r", g=4).unsqueeze(2) \
            .broadcast_to([128, 4, KNN, 16])

    def s_tail(b):
        # W2 layers: E += B_i ; h = prelu(E + b_in) ; z2 = W2 @ h
        E = E_t.pop(b)
        ev = E.rearrange("c (g k r) -> c g k r", g=4, k=KNN)
        nc.vector.tensor_tensor(out=ev, in0=ev, in1=bt_slice(b), op=ALU.add)
        h = pool.tile([128, 64 * KNN], FP32, tag="h", bufs=2,
                      name=f"h_{lname}_{b}")
        h_t[b] = h
        nc.scalar.activation(out=h[:], in_=E, func=ACT.Prelu,
                             alpha=0.2, bias=b_in2)
        z2 = zpool.tile([128, 64 * KNN], FP32, tag="z2",
                        name=f"z2_{lname}_{b}")
        z2_t[b] = z2
        for csl in (slice(0, 512), slice(512, 1024), slice(1024, 1280)):
            nc.tensor.matmul(z2[:, csl], w2T2, h[:, csl], start=True,
                             stop=True)

    def s_red(b):
        # max over k (outer lrelu is monotone -> prelu after the reduce)
        z2 = z2_t.pop(b)
        h_t.pop(b, None)
        yd = pool.tile([128, 64], FP32, tag="yd", bufs=2,
                       name=f"yd_{lname}_{b}")
        nc.vector.tensor_reduce(
            out=yd[:].rearrange("c (g r) -> c g r", g=4),
            in_=z2[:].rearrange("c (g k r) -> c g r k", g=4, k=KNN),
            axis=mybir.AxisListType.X, op=ALU.max)
        _emit_y(b, yd)

    def s_tail3(b):
        # layer 3: max_k (A_j + B_i) = (max_k A_j) + B_i
        E = E_t.pop(b)
        yd = pool.tile([128, 64], FP32, tag="yd", bufs=2,
                       name=f"yd_{lname}_{b}")
        nc.vector.tensor_reduce(
            out=yd[:].rearrange("c (g r) -> c g r", g=4),
            in_=E.rearrange("c (g k r) -> c g r k", g=4, k=KNN),
            axis=mybir.AxisListType.X, op=ALU.max)
        nc.vector.tensor_tensor(
            out=yd[:].rearrange("c (g r) -> c g r", g=4),
            in0=yd[:].rearrange("c (g r) -> c g r", g=4),
            in1=Bt2[:, b * 64:(b + 1) * 64]
            .rearrange("c (g r) -> c g r", g=4), op=ALU.add)
        _emit_y(b, yd)

    def _emit_y(b, yd):
        if xmy2_next is not None:
            ydst = xmy2_next[:, b * 64:(b + 1) * 64]
        else:
            ydst = pool.tile([128, 64], FP32, tag="yd2", bufs=2,
                             name=f"yd2_{lname}_{b}")
        nc.scalar.activation(out=ydst, in_=yd[:], func=ACT.Prelu,
                             alpha=0.2, bias=b_out2)
        nc.sync.dma_start(y_full[0:64, b * 128:b * 128 + 64], ydst[0:64, :])
        nc.sync.dma_start(y_full[0:64, b * 128 + 64:b * 128 + 128],
                          ydst[64:128, :])

    # stage-shifted emission: pd(t+1), topk(t), gather-group when its last
    # block's indices land, tail(t-GGRP), red(t-GGRP-1)
    s_pd(0)
    if w2T2 is not None:
        for t in range(NBLK + 2 * GGRP):
            if t + 1 < NBLK:
                s_pd(t + 1)
            if t < NBLK:
                s_topk(t)
                if t % GGRP == GGRP - 1:
                    s_gth(t // GGRP)
            if GGRP <= t < NBLK + GGRP:
                s_tail(t - GGRP)
            if GGRP + 1 <= t < NBLK + GGRP + 1:
                s_red(t - GGRP - 1)
    else:
        for t in range(NBLK + 2 * GGRP):
            if t + 1 < NBLK:
                s_pd(t + 1)
            if t < NBLK:
                s_topk(t)
                if t % GGRP == GGRP - 1:
                    s_gth(t // GGRP)
            if GGRP <= t < NBLK + GGRP:
                s_tail3(t - GGRP)
    lctx.close()


def build(pairs, reps=1, dbg=False):
    """Build + compile the SPMD program. pairs: replica groups (list of lists).
    reps: run the whole pipeline this many times (for slope-based timing)."""
    nc = bacc.Bacc("TRN2", target_bir_lowering=False, debug=False)
    DBG = nc.dram_tensor("dbg", [64, HALF], FP32, kind="ExternalOutput") \
        if dbg else None

    def din(name, shape, dtype):
        return nc.dram_tensor(name, shape, dtype, kind="ExternalInput")

    X = din("x_full", [C0, N], FP32)              # holds 2x
    XMY = din("xmy_aug", [C0 + 1, HALF], FP32)    # [x_my; ones] (unscaled)
    XMY2 = din("xmy2_l1", [2 * C0, HALF // 2], FP32)  # dual-half (unscaled)
    WS = din("wpack", [128, WPACK_W], FP32)
    BIASES = din("biases", [128, BIAS_W], FP32)
    W6T3 = din("w6T3", [64, 3072], F32R)
    W7XT3 = din("w7xT3", [64, 1536], F32R)
    W7GT8 = din("w7gT8", [128, 4096], FP32)
    W8T4 = din("w8T4", [128, 1024], F32R)
    W9T2 = din("w9T2", [128, 16], F32R)
    OUT = nc.dram_tensor("out", [8, HALF], FP32, kind="ExternalOutput")

    with tile.TileContext(nc) as tc:
        ctx = contextlib.ExitStack()
        persist = ctx.enter_context(tc.tile_pool(name="persist", bufs=1))
        ppool = ctx.enter_context(tc.tile_pool(name="ps", bufs=3, space="PSUM"))
        dpool = ctx.enter_context(tc.tile_pool(name="dram", bufs=1, space="DRAM"))

        wpack = persist.tile([128, WPACK_W], FP32, name="wpack")
        nc.sync.dma_start(wpack[:], WS[:])

        def wsl(name):
            o, r, c = WPACK[name]
            return wpack[0:r, o:o + c]

        def ones_col(Cin):
            o, _, _ = WPACK["ones"]
            return wpack[0:Cin, o:o + 1]

        biases = persist.tile([128, BIAS_W], FP32, name="biases")
        nc.sync.dma_start(biases[:], BIASES[:])

        def bsl(name, p=128):
            o, w = BIAS_LAYOUT[name]
            return biases[0:p, o:o + w]

        x1my = persist.tile([65, HALF], FP32, name="x1my")
        x2my = persist.tile([65, HALF], FP32, name="x2my")
        x3my = persist.tile([64, HALF], FP32, name="x3my")
        xmy2b = persist.tile([128, HALF // 2], FP32, name="xmy2b")
        xmy2l1 = persist.tile([2 * C0, HALF // 2], FP32, name="xmy2l1")
        xf = persist.tile([65, N], FP32, name="xf")

        for _rep in range(reps):
            nc.sync.dma_start(xf[0:C0, :], X[:])
            nc.sync.dma_start(x2my[0:C0 + 1, :], XMY[:])
            nc.sync.dma_start(x2my[64:65, :], XMY[C0:C0 + 1, :])
            nc.sync.dma_start(x1my[64:65, :], XMY[C0:C0 + 1, :])
            nc.sync.dma_start(xmy2l1[:], XMY2[:])
            ectx = contextlib.ExitStack()
            pool = ectx.enter_context(tc.tile_pool(name="work", bufs=1))
            pools = (pool, ppool)

            def ag_half(half_ap, full_ap, name):
                """Stage 2*half_ap -> pair-AllGather -> full_ap rows 0:64."""
                st = pool.tile([64, HALF], FP32, tag="agst", name=f"ags_{name}")
                nc.scalar.activation(out=st[:], in_=half_ap, func=ACT.Copy,
                                     scale=2.0)
                bi = dpool.tile([64, HALF], FP32, name=f"agi_{name}")
                nc.sync.dma_start(bi[:], st[:])
                if pairs is None:  # timing-sim variant: fake the exchange
                    nc.sync.dma_start(full_ap[0:64, 0:HALF], bi[:])
                    nc.sync.dma_start(full_ap[0:64, HALF:N], bi[:])
                    return
                bo = dpool.tile([2, 64, HALF], FP32, name=f"ago_{name}")
                nc.gpsimd.collective_compute("AllGather", ALU.bypass,
                                             replica_groups=pairs,
                                             ins=[bi[:]], outs=[bo[:]])
                nc.sync.dma_start(full_ap[0:64, 0:HALF], bo[0])
                nc.sync.dma_start(full_ap[0:64, HALF:N], bo[1])

            # ---- layer 1 ----
            _edge_layer(nc, tc, pools, C0, x2my[0:C0 + 1, :], xmy2l1, xf,
                        wsl("wnW1"), wsl("bw2_1"), ones_col(C0),
                        bsl("b1"), wsl("w2T2"), bsl("b2"),
                        xmy2b, x1my, "l1")
            if dbg:
                nc.sync.dma_start(DBG[:], x1my[0:64, :].bitcast(FP32))
            ag_half(x1my[0:64, :], xf, "x1")

            # ---- layer 2 ----
            _edge_layer(nc, tc, pools, 64, x1my, xmy2b, xf, wsl("wnW3"),
                        wsl("bw2_3"), ones_col(64), bsl("b3"), wsl("w4T2"),
                        bsl("b4"), xmy2b, x2my, "l2")
            ag_half(x2my[0:64, :], xf, "x2")

            # ---- layer 3 ----
            _edge_layer(nc, tc, pools, 64, x2my, xmy2b, xf, wsl("wnW5"),
                        wsl("bw2_5"), ones_col(64), None, None,
                        bsl("b5"), None, x3my, "l3")
            ectx.close()

            # ---- head ----
            hctx = contextlib.ExitStack()
            hpool = hctx.enter_context(tc.tile_pool(name="head", bufs=1))
            w6T3 = hpool.tile([64, 3072], F32R, name="w6t")
            nc.sync.dma_start(w6T3[:], W6T3[:])
            w7xT3 = hpool.tile([64, 1536], F32R, name="w7xt")
            nc.sync.dma_start(w7xT3[:], W7XT3[:])
            w7gT8 = hpool.tile([128, 4096], FP32, name="w7gt")
            nc.sync.dma_start(w7gT8[:], W7GT8[:])
            w8T4 = hpool.tile([128, 1024], F32R, name="w8t")
            nc.sync.dma_start(w8T4[:], W8T4[:])
            w9T2 = hpool.tile([128, 16], F32R, name="w9t")
            nc.sync.dma_start(w9T2[:], W9T2[:])

            catsr = []
            for j, src_t in enumerate([x1my, x2my, x3my]):
                cr = hpool.tile([64, HALF], F32R, name=f"catr_{j}")
                nc.scalar.activation(out=cr[:], in_=src_t[0:64, :],
                                     func=ACT.Copy)
                catsr.append(cr)
            cats = catsr  # rows 0:64 each, f32r for the head matmuls

            # y6max[p, m] = max_n (W6 @ cat)[m*128+p, n]
            y6max = hpool.tile([128, 8], FP32, name="y6max")
            for m in range(8):
                y6p = hpool.tile([128, 4], FP32, tag="y6p", bufs=2, name=f"y6p_{m}")
                for nch in range(4):
                    sl = slice(nch * 512, (nch + 1) * 512)
                    pp = ppool.tile([128, 512], FP32, tag="mm", name=f"z6_{m}_{nch}")
                    for j in range(3):
                        nc.tensor.matmul(pp[:], w6T3[:, j * 1024 + m * 128:
                                                     j * 1024 + (m + 1) * 128],
                                         cats[j][0:64, sl], start=(j == 0),
                                         stop=(j == 2))
                    nc.vector.tensor_reduce(out=y6p[:, nch:nch + 1],
                                            in_=pp[:], axis=mybir.AxisListType.X,
                                            op=ALU.max)
                nc.vector.tensor_reduce(out=y6max[:, m:m + 1],
                                        in_=y6p[:], axis=mybir.AxisListType.X,
                                        op=ALU.max)
            # pair AllReduce(max), then leaky-relu
            gb_i = dpool.tile([128, 8], FP32, name="ar_i")
            gb_o = dpool.tile([128, 8], FP32, name="ar_o")
            nc.sync.dma_start(gb_i[:], y6max[:])
            if pairs is not None:
                nc.gpsimd.collective_compute("AllReduce", ALU.max,
                                             replica_groups=pairs,
                                             ins=[gb_i[:]], outs=[gb_o[:]])
            else:
                nc.sync.dma_start(gb_o[:], gb_i[:])
            gmxpre = hpool.tile([128, 8], FP32, name="gmxpre")
            nc.sync.dma_start(gmxpre[:], gb_o[:])
            gmx = hpool.tile([128, 8], FP32, name="gmx")
            for m in range(8):
                nc.scalar.activation(out=gmx[:, m:m + 1], in_=gmxpre[:, m:m + 1],
                                     func=ACT.Prelu, alpha=0.2,
                                     bias=bsl("b6", 128)[:, m:m + 1])

            # b7eff = W7g @ gmx + b7
            b7e = hpool.tile([128, 4], FP32, name="b7e")
            for m in range(4):
                pw = ppool.tile([128, 512], FP32, tag="mm", name=f"w7g_{m}")
                for k in range(8):
                    nc.tensor.matmul(pw[:, 0:1],
                                     w7gT8[:, k * 512 + m * 128:k * 512 + (m + 1) * 128],
                                     gmx[:, k:k + 1], start=(k == 0), stop=(k == 7))
                nc.scalar.activation(out=b7e[:, m:m + 1], in_=pw[:, 0:1],
                                     func=ACT.Identity,
                                     bias=bsl("b7", 128)[:, m:m + 1])

            # h7 = lrelu(W7x @ cat + b7e)  [128, 8192] = 4 m-chunks x 2048
            h7 = hpool.tile([128, 8192], F32R, name="h7")
            for m in range(4):
                for nch in range(4):
                    sl = slice(nch * 512, (nch + 1) * 512)
                    osl = slice(m * 2048 + nch * 512, m * 2048 + (nch + 1) * 512)
                    pp = ppool.tile([128, 512], FP32, tag="mm", name=f"z7_{m}_{nch}")
                    for j in range(3):
                        nc.tensor.matmul(pp[:], w7xT3[:, j * 512 + m * 128:
                                                      j * 512 + (m + 1) * 128],
                                         cats[j][0:64, sl], start=(j == 0),
                                         stop=(j == 2))
                    nc.scalar.activation(out=h7[:, osl], in_=pp[:],
                                         func=ACT.Prelu, alpha=0.2,
                                         bias=b7e[:, m:m + 1])

            # h8 = lrelu(W8 @ h7 + b8)  [128, 4096] = 2 m-chunks x 2048
            h8 = hpool.tile([128, 4096], F32R, name="h8")
            for m8 in range(2):
                for nch in range(4):
                    osl = slice(m8 * 2048 + nch * 512, m8 * 2048 + (nch + 1) * 512)
                    pp = ppool.tile([128, 512], FP32, tag="mm", name=f"z8_{m8}_{nch}")
                    for k in range(4):
                        ksl = slice(k * 2048 + nch * 512, k * 2048 + (nch + 1) * 512)
                        lhs = w8T4[:, k * 256 + m8 * 128:k * 256 + (m8 + 1) * 128]
                        nc.tensor.matmul(pp[:], lhs, h7[:, ksl], start=(k == 0),
                                         stop=(k == 3))
                    nc.scalar.activation(out=h8[:, osl], in_=pp[:],
                                         func=ACT.Prelu, alpha=0.2,
                                         bias=bsl("b8", 128)[:, m8:m8 + 1])

            # out = W9 @ h8
            outsb = hpool.tile([8, HALF], FP32, name="outsb")
            for nch in range(4):
                sl = slice(nch * 512, (nch + 1) * 512)
                pp = ppool.tile([128, 512], FP32, tag="mm", name=f"z9_{nch}")
                for k2 in range(2):
                    ksl = slice(k2 * 2048 + nch * 512, k2 * 2048 + (nch + 1) * 512)
                    lhs = w9T2[:, k2 * 8:(k2 + 1) * 8]
                    nc.tensor.matmul(pp[0:8, :], lhs, h8[:, ksl], start=(k2 == 0),
                                     stop=(k2 == 1))
                nc.scalar.activation(out=outsb[:, sl], in_=pp[0:8, :], func=ACT.Copy)
            nc.sync.dma_start(OUT[:], outsb[:])
            hctx.close()
        ctx.close()

    nc.compile()
    return nc


def _prep_in_maps(x, W1, W2, W3, W4, W5, W6, W7, W8, W9,
                  g1, b1, g2, b2, g3, b3, g4, b4, g5, b5, g6, b6, g7, b7, g8, b8):
    f = np.float32
    sc = {i: (g / np.sqrt(f(1.0) + f(EPS))).astype(f) for i, g in
          [(1, g1), (2, g2), (3, g3), (4, g4), (5, g5), (6, g6), (7, g7), (8, g8)]}

    def fold(W, s):
        return (W * s[:, None]).astype(f)

    W1f = fold(W1, sc[1]); W2f = fold(W2, sc[2]); W3f = fold(W3, sc[3])
    W4f = fold(W4, sc[4]); W5f = fold(W5, sc[5]); W6f = fold(W6, sc[6])
    W7f = fold(W7, sc[7]); W8f = fold(W8, sc[8])

    def edge_w(Wf, Cin):
        wn = Wf[:, :Cin]
        bw = Wf[:, Cin:] - wn
        # xf holds 2x the features -> fold the 1/2 into wn
        wnT = np.ascontiguousarray(wn.T) * f(0.5)
        bwT = np.ascontiguousarray(bw.T)
        wnW = np.concatenate([wnT, wnT], axis=1)            # [Cin, 128]
        bw2 = np.zeros((2 * Cin, 128), f)                   # block-diag
        bw2[0:Cin, 0:64] = bwT
        bw2[Cin:2 * Cin, 64:128] = bwT
        return wnW, bw2

    def bdiag(Wf):
        t = np.zeros((128, 128), f)
        t[0:64, 0:64] = Wf.T
        t[64:128, 64:128] = Wf.T
        return t

    wnW1, bw2_1 = edge_w(W1f, C0)
    wnW3, bw2_3 = edge_w(W3f, 64)
    wnW5, bw2_5 = edge_w(W5f, 64)

    wpack = np.zeros((128, WPACK_W), f)
    parts = dict(wnW1=wnW1, bw2_1=bw2_1, w2T2=bdiag(W2f), wnW3=wnW3,
                 bw2_3=bw2_3, w4T2=bdiag(W4f), wnW5=wnW5, bw2_5=bw2_5,
                 ones=np.ones((64, 1), f))
    for nm, (o, r, c) in WPACK.items():
        p = parts[nm]
        assert p.shape == (r, c), (nm, p.shape)
        wpack[0:r, o:o + c] = p

    biases = np.zeros((128, BIAS_W), f)
    for nm, bvec in [("b1", b1), ("b2", b2), ("b3", b3), ("b4", b4),
                     ("b5", b5)]:
        o, w = BIAS_LAYOUT[nm]
        biases[0:64, o] = bvec.astype(f)
        biases[64:128, o] = bvec.astype(f)
    for nm, bvec in [("b6", b6), ("b7", b7), ("b8", b8)]:
        o, w = BIAS_LAYOUT[nm]
        bm = bvec.astype(f).reshape(w, -1).T  # [p, w]
        biases[0:bm.shape[0], o:o + w] = bm

    W6T = W6f.T
    w6T3 = np.concatenate([W6T[0:64], W6T[64:128], W6T[128:192]], axis=1)
    W7g = W7f[:, :1024]; W7x = W7f[:, 1024:]
    W7xT = W7x.T
    w7xT3 = np.concatenate([W7xT[0:64], W7xT[64:128], W7xT[128:192]], axis=1)
    W7gT = W7g.T
    w7gT8 = np.concatenate([W7gT[k * 128:(k + 1) * 128] for k in range(8)], axis=1)
    W8T = W8f.T
    w8T4 = np.concatenate([W8T[k * 128:(k + 1) * 128] for k in range(4)], axis=1)
    W9T = W9.astype(f).T
    w9T2 = np.concatenate([W9T[0:128], W9T[128:256]], axis=1)

    com = dict(wpack=wpack, biases=biases,
               w6T3=np.ascontiguousarray(w6T3),
               w7xT3=np.ascontiguousarray(w7xT3),
               w7gT8=np.ascontiguousarray(w7gT8),
               w8T4=np.ascontiguousarray(w8T4),
               w9T2=np.ascontiguousarray(w9T2))

    in_maps = []
    for c in range(2 * B):
        s, h = c // 2, c % 2
        xs = np.asarray(x[s], dtype=f)
        xmy = xs[:, h * HALF:(h + 1) * HALF]
        xmy_aug = np.concatenate([xmy, np.ones((1, HALF), f)], axis=0)
        # dual-half layout: [0:C0] = rows 0:64 of each block, [C0:2C0] = 64:128
        xmy2 = np.empty((2 * C0, HALF // 2), f)
        xv = xmy.reshape(C0, NBLK, 2, 64)
        xmy2[0:C0] = xv[:, :, 0, :].reshape(C0, -1)
        xmy2[C0:2 * C0] = xv[:, :, 1, :].reshape(C0, -1)
        m = dict(com)
        m["x_full"] = np.ascontiguousarray(xs * f(2.0))
        m["xmy_aug"] = np.ascontiguousarray(xmy_aug)
        m["xmy2_l1"] = np.ascontiguousarray(xmy2)
        in_maps.append(m)
    return in_maps


def _build_executor(nc, n_cores):
    """Cached jitted PJRT executor (run_bass_kernel_spmd re-lowers per call)."""
    import jax
    from jax.sharding import Mesh, PartitionSpec
    from jax.experimental.shard_map import shard_map
    from concourse.bass2jax import (
        install_neuronx_cc_hook, _bass_exec_p, partition_id_tensor)

    install_neuronx_cc_hook()
    partition_name = (nc.partition_id_tensor.name
                      if nc.partition_id_tensor else None)
    in_names, out_names, out_avals, zero_shapes = [], [], [], []
    for alloc in nc.m.functions[0].allocations:
        if not isinstance(alloc, mybir.MemoryLocationSet):
            continue
        name = alloc.memorylocations[0].name
        if alloc.kind == "ExternalInput":
            if name != partition_name:
                in_names.append(name)
        elif alloc.kind == "ExternalOutput":
            shape = tuple(alloc.tensor_shape)
            dtype = mybir.dt.np(alloc.dtype)
            out_names.append(name)
            out_avals.append(jax.core.ShapedArray(shape, dtype))
            zero_shapes.append((shape, dtype))
    n_params = len(in_names)
    n_outs = len(out_avals)
    all_names = in_names + out_names
    if partition_name is not None:
        all_names.append(partition_name)

    def _body(*args):
        operands = list(args)
        if partition_name is not None:
            operands.append(partition_id_tensor())
        return tuple(_bass_exec_p.bind(
            *operands, out_avals=tuple(out_avals), in_names=tuple(all_names),
            out_names=tuple(out_names), lowering_input_output_aliases=(),
            sim_require_finite=True, sim_require_nnan=True, nc=nc))

    devices = jax.devices()[:n_cores]
    mesh = Mesh(np.asarray(devices), ("core",))
    in_specs = (PartitionSpec("core"),) * (n_params + n_outs)
    out_specs = (PartitionSpec("core"),) * n_outs
    donate = tuple(range(n_params, n_params + n_outs))
    fn = jax.jit(shard_map(_body, mesh=mesh, in_specs=in_specs,
                           out_specs=out_specs, check_rep=False),
                 donate_argnums=donate, keep_unused=True)

    def run(in_maps):
        concat_in = [np.concatenate([np.asarray(in_maps[c][nm])
                                     for c in range(n_cores)], axis=0)
                     for nm in in_names]
        zeros = [np.zeros((n_cores * s[0], *s[1:]), d) for s, d in zero_shapes]
        outs = fn(*concat_in, *zeros)
        return [{nm: np.asarray(outs[i]).reshape(n_cores, *out_avals[i].shape)[c]
                 for i, nm in enumerate(out_names)} for c in range(n_cores)]

    return run


def kernel(**inputs):
    inputs = {k: np.asarray(v, dtype=np.float32) for k, v in inputs.items()}
    if "nc" not in _CACHE:
        _CACHE["nc"] = build([[0, 1], [2, 3], [4, 5], [6, 7]])
        _CACHE["run"] = _build_executor(_CACHE["nc"], 2 * B)
    in_maps = _prep_in_maps(**inputs)
    results = _CACHE["run"](in_maps)
    out = np.empty((B, 8, N), dtype=np.float32)
    for c in range(2 * B):
        s, h = c // 2, c % 2
        out[s, :, h * HALF:(h + 1) * HALF] = results[c]["out"]
    return out
